# revision 12
# baseline (speedup 1.0000x reference)
"""Trainium2 Bass kernel for nn_Attention_730144440595 (NormAttention block).

8 NeuronCores, data-parallel over batch (16 -> 2/core). Per core, vs the
previous version:
  - x stays UNcentered; the channel-mean subtraction is a rank-1 (K=1)
    correction matmul folded into the QKV / V^T GEMMs (host-precomputed
    column sums of the weights).
  - q,k packed two heads per 128-row tile; sims are K=64 matmuls row-tiled
    onto the upper/lower halves of the PE array so a head pair runs
    concurrently.
  - V^T computed directly on the PE (x-tile stationary) instead of DMA
    transposes; rstd is folded into V at evacuation; the denominator comes
    from an all-ones column block so softmax division is a fused
    reciprocal-multiply.
  - attention weights et = exp(logits) are stored as fp8e4 and attn@V runs
    in DoubleRow perf mode (2 fp8 MACs/cell/cycle, K=256 per pass).
  - exp is split across engines: one head of each pair uses the Scalar
    engine's exact Exp (fp8 out); the other uses a Schraudolph bit-trick
    exp on the Vector engine (custom DVE op: uint8(relu(A*logit+B)) whose
    bits ARE the fp8 value).
"""

import sys
import types

import numpy as np

B = 2
C = 256
N = 1024
HEADS = 4
D = 64
P = 128
NCORES = 8
LN_EPS = 1e-5
LOG2E = float(1.0 / np.log(2.0))
A8 = 8.0 * LOG2E                      # schraudolph slope per logit
CS = -2.7                             # schraudolph logit shift
B8P = 56.0 + 1.0 + A8 * CS            # schraudolph intercept (trunc-calibrated)
CE = float(np.log(128.0) - 8.0)       # exact-exp logit shift (et <= ~141)
LOG8 = float(np.log(8.0))
LOG8A8 = float(np.log(8.0 * A8))
RC0, RC1 = -0.23549792, 2.0017324     # 1-NR reciprocal seed consts


def _host_consts():
    cst = np.zeros((P, 16), np.float32)
    for cc in range(2):
        for p in range(P):
            cst[p, 4 * cc + 2 * cc + p // 64] = 1.0   # E_ind cols 0-3 / 4-7
    cst[:, 8] = 1.0 / 256.0    # rhs_x col0 (mean)
    cst[:, 11] = 1.0 / 256.0   # rhs_q col1 (msq)
    cst[:, 12] = -1.0 / 256.0  # negones
    cst[:64, 13] = 1.0         # khalf0
    cst[64:, 14] = 1.0         # khalf1
    cst4 = np.zeros((HEADS, 2 * P), np.float32)
    for cc in range(2):
        for m in range(P):
            cst4[2 * cc + m // 64, cc * P + m] = 1.0  # E4
    return cst, cst4


def _host_weights(w_qkv, w_out, g):
    import ml_dtypes
    wg = np.asarray(w_qkv, np.float32) * np.asarray(g, np.float32).reshape(1, C)
    wqk = np.ascontiguousarray(wg[0:512].T)           # [C, 512] (q 0-255, k 256-511)
    wvT = np.ascontiguousarray(wg[512:768].T)         # [C, 256]
    wsqk = wqk.sum(0, keepdims=True)                  # [1, 512]
    wvs = wvT.sum(0, keepdims=True)                   # [1, 256]
    wo = np.asarray(w_out, np.float32).T.reshape(HEADS, D, C)
    wot = np.ascontiguousarray(wo.transpose(1, 0, 2)).reshape(D, HEADS * C)
    bf = ml_dtypes.bfloat16
    return (wqk.astype(bf), wvT.astype(bf), wsqk.astype(bf), wvs.astype(bf),
            wot.astype(bf))


def _install_ntff_hook():
    try:
        import antenv
        if getattr(antenv, "axon_hooks", None) is not None:
            return
        from trn_agent_boot.trn_boot import _ntff_profile_via_ctypes
        hook = _ntff_profile_via_ctypes('/opt/axon/libaxon_pjrt.so')
        mod = types.ModuleType('antenv.axon_hooks')
        mod._hook = hook
        mod.get_axon_ntff_profile_hook = lambda: mod._hook
        mod.set_axon_ntff_profile_hook = lambda h: setattr(mod, '_hook', h)
        sys.modules['antenv.axon_hooks'] = mod
        antenv.axon_hooks = mod
    except Exception:
        pass


def _register_dve_ops():
    """Register SCH_EXP8 and RECIP1_MUL custom DVE ops (idempotent)."""
    from concourse import dve_ops as DO
    from concourse.dve_spec import (Spec, Src0, Src1, C0, C1, Bin, AluOp,
                                    lower, relu, _has_src1)
    from concourse.dve_uop import DveOpSpec

    def _reg(name, spec):
        if name in DO._SUB_OPCODE_FOR_NAME:
            return next(o for o in DO.OPS if o.name == name)
        shas = {}
        for ver in ("v3", "v4"):
            tmp = DveOpSpec(name=name, opcode=1 + len(DO.OPS),
                            uops=lower(spec, ver=ver), rd1_en=_has_src1(spec))
            shas[ver] = tmp.sha(ver)
        op = DO.DveOp(name, spec, subdim=False, uops_sha=shas)
        DO.OPS.append(op)
        DO.CUSTOM_DVE_SPECS[op.name] = op.spec
        DO._SUB_OPCODE_FOR_NAME[op.name] = DO._CUSTOM_DVE_ROW_BASE + len(DO.OPS) - 1
        return op

    def _ref_sch8(in0, in1, s0, s1, imm2):
        return np.maximum(in0 * s0 + s1, 0.0)

    sch8 = _reg("SCH_EXP8", Spec(body=relu(Src0 * C0 + C1), reference=_ref_sch8))

    def _ref_recip1_mul(in0, in1, s0, s1, imm2):
        not_x = (~in0.view(np.int32)).view(np.float32)
        y0 = not_x * s0
        y1 = y0 * (s1 - in0 * y0)
        return in1 * y1

    _not = Bin(AluOp.BITWISE_NOT, Src0, Src0)
    _y0 = _not * C0
    rcm = _reg("RECIP1_MUL",
               Spec(body=Src1 * (_y0 * (C1 - Src0 * _y0)),
                    reference=_ref_recip1_mul))
    return sch8, rcm


def build_nc():
    import concourse.bass as bass
    import concourse.tile as tile
    import concourse.mybir as mybir
    from concourse import bacc
    from contextlib import ExitStack

    SCH_EXP8, RECIP1_MUL = _register_dve_ops()

    dt = mybir.dt
    f32 = dt.float32
    bf16 = dt.bfloat16
    fp8 = dt.float8e4
    u8 = dt.uint8
    AF = mybir.ActivationFunctionType
    OP = mybir.AluOpType
    DR = mybir.MatmulPerfMode.DoubleRow

    # Keep Exp/Ln only in the combined table set (avoid ACT table thrash).
    from concourse.hw_specs import get_activation_tables
    _tabs = get_activation_tables("gen3")
    for _name, _fns in _tabs.items():
        if _name != "natural_log_exp_and_others":
            _fns.discard(AF.Exp)
            _fns.discard(AF.Ln)

    nc = bacc.Bacc("TRN2", target_bir_lowering=False, num_devices=NCORES)
    x_d = nc.dram_tensor("x", [B, C, N], f32, kind="ExternalInput").ap()
    xbf_d = nc.dram_tensor("xbf", [B, C, N], bf16, kind="ExternalInput").ap()
    wqk_d = nc.dram_tensor("wqk", [C, 512], bf16, kind="ExternalInput").ap()
    wvT_d = nc.dram_tensor("wvT", [C, 256], bf16, kind="ExternalInput").ap()
    wsqk_d = nc.dram_tensor("wsqk", [1, 512], bf16, kind="ExternalInput").ap()
    wvs_d = nc.dram_tensor("wvs", [1, 256], bf16, kind="ExternalInput").ap()
    wot_d = nc.dram_tensor("wot", [D, HEADS * C], bf16, kind="ExternalInput").ap()
    cst_d = nc.dram_tensor("cst", [P, 16], f32, kind="ExternalInput").ap()
    cst4_d = nc.dram_tensor("cst4", [HEADS, 2 * P], f32, kind="ExternalInput").ap()
    out_d = nc.dram_tensor("out", [B, C, N], f32, kind="ExternalOutput").ap()

    with tile.TileContext(nc) as tc, ExitStack() as ctx:
        const = ctx.enter_context(tc.tile_pool(name="const", bufs=1))
        big = ctx.enter_context(tc.tile_pool(name="big", bufs=1))
        tmp = ctx.enter_context(tc.tile_pool(name="tmp", bufs=2))
        etp = ctx.enter_context(tc.tile_pool(name="etp", bufs=2))
        outp = ctx.enter_context(tc.tile_pool(name="outp", bufs=2))
        psS = ctx.enter_context(tc.tile_pool(name="psS", bufs=4, space="PSUM"))
        psF = ctx.enter_context(tc.tile_pool(name="psF", bufs=1, space="PSUM"))
        psU = ctx.enter_context(tc.tile_pool(name="psU", bufs=1, space="PSUM"))

        def mm(out, lhsT, rhs, start, stop, **kw):
            nc.tensor.matmul(out, lhsT, rhs, start=start, stop=stop, **kw)

        # ---------------- constants ----------------
        cst_f = tmp.tile([P, 16], f32, tag="cst_f", name="cst_f")
        nc.sync.dma_start(cst_f, cst_d[:])
        cst = const.tile([P, 16], bf16, tag="cst", name="cst")
        nc.vector.tensor_copy(out=cst[:], in_=cst_f[:])
        E_ind = [cst[:, 0:4], cst[:, 4:8]]
        rhs_x = cst[:, 8:10]
        rhs_q = cst[:, 10:12]
        negones = cst[:, 12:13]
        khalf = [cst[:, 13:14], cst[:, 14:15]]
        cst4_f = tmp.tile([HEADS, 2 * P], f32, tag="cst4_f", name="cst4_f")
        nc.sync.dma_start(cst4_f, cst4_d[:])
        cst4 = const.tile([HEADS, 2 * P], bf16, tag="cst4", name="cst4")
        nc.vector.tensor_copy(out=cst4[:], in_=cst4_f[:])
        E4 = [cst4[:, 0:128], cst4[:, 128:256]]

        eps_col = const.tile([P, 1], f32, tag="eps_col", name="eps_col")
        nc.vector.memset(eps_col[:], LN_EPS)
        ce_col = const.tile([P, 1], f32, tag="ce_col", name="ce_col")
        nc.vector.memset(ce_col[:], CE)
        log8_col = const.tile([P, 1], f32, tag="log8_col", name="log8_col")
        nc.vector.memset(log8_col[:], LOG8)
        log8a8_col = const.tile([P, 1], f32, tag="log8a8_col", name="log8a8_col")
        nc.vector.memset(log8a8_col[:], LOG8A8)

        # ---------------- weight / input loads ----------------
        wqk_sb = big.tile([P, 2, 512], bf16, tag="wqk", name="wqk_sb")
        nc.sync.dma_start(wqk_sb, wqk_d.rearrange("(cc p) o -> p cc o", p=P))
        wvT_sb = big.tile([P, 2, 256], bf16, tag="wvT", name="wvT_sb")
        nc.sync.dma_start(wvT_sb, wvT_d.rearrange("(cc p) o -> p cc o", p=P))
        wsqk_sb = big.tile([1, 512], bf16, tag="wsqk", name="wsqk_sb")
        nc.sync.dma_start(wsqk_sb, wsqk_d[:])
        wvs_sb = big.tile([1, 256], bf16, tag="wvs", name="wvs_sb")
        nc.sync.dma_start(wvs_sb, wvs_d[:])
        wot_sb = big.tile([D, HEADS, C], bf16, tag="wot", name="wot_sb")
        nc.sync.dma_start(wot_sb, wot_d.rearrange("d (h c) -> d h c", h=HEADS))

        x_sb = [[big.tile([P, N], f32, tag=f"x{b}{cc}", name=f"x{b}{cc}")
                 for cc in range(2)] for b in range(B)]
        x_bf = [[big.tile([P, N], bf16, tag=f"xbf{b}{cc}", name=f"xbf{b}{cc}")
                 for cc in range(2)] for b in range(B)]
        for b in range(B):
            for cc in range(2):
                nc.sync.dma_start(x_bf[b][cc], xbf_d[b, cc * P:(cc + 1) * P, :])
                nc.sync.dma_start(x_sb[b][cc], x_d[b, cc * P:(cc + 1) * P, :])

        # vaug: [128 j, 4 pair, 2 slot, 4 head, 128 m] fp8; m 0-63 = v*rstd,
        # m 64-127 = ones (denominator block). ones written once.
        vaug = [big.tile([P, 4, 2, HEADS, P], fp8, tag=f"va{b}", name=f"va{b}")
                for b in range(B)]
        for b in range(B):
            nc.gpsimd.memset(vaug[b][:, :, :, :, D:P], 1.0)

        # q/k tiles packed 2 heads per tile; u tiles [64, 1024]
        q_sb = [[big.tile([P, N], bf16, tag=f"q{b}{cc}", name=f"q{b}{cc}")
                 for cc in range(2)] for b in range(B)]
        k_sb = [[big.tile([P, N], bf16, tag=f"k{b}{cc}", name=f"k{b}{cc}")
                 for cc in range(2)] for b in range(B)]
        u_sb = [[big.tile([D, 2, N], bf16, tag=f"u{b}{cc}", name=f"u{b}{cc}")
                 for cc in range(2)] for b in range(B)]

        rstd = [big.tile([P, 8], f32, tag=f"rstd{b}", name=f"rstd{b}") for b in range(B)]
        negmu = [big.tile([1, N], bf16, tag=f"nmu{b}", name=f"nmu{b}") for b in range(B)]
        b8e = [big.tile([P, 8, HEADS], f32, tag=f"b8e{b}", name=f"b8e{b}") for b in range(B)]
        b8s = [big.tile([P, 8, HEADS], f32, tag=f"b8s{b}", name=f"b8s{b}") for b in range(B)]
        a_sb = [big.tile([HEADS, N], bf16, tag=f"a{b}", name=f"a{b}") for b in range(B)]
        u_ps = {}

        # ---------------- phases ----------------
        def _stats_mm(b):
            xsq = [tmp.tile([P, N], bf16, tag=f"xsq{cc}", name=f"xsq{b}{cc}")
                   for cc in range(2)]
            for cc in range(2):
                nc.gpsimd.tensor_mul(xsq[cc][:], x_bf[b][cc][:], x_bf[b][cc][:])
            st_ps = psF.tile([P, 8, 2], f32, tag="F", name="st_ps")
            for ic in range(8):
                sl = st_ps[:, ic]
                mm(sl, x_bf[b][0][:, ic * P:(ic + 1) * P], rhs_x, True, False)
                mm(sl, x_bf[b][1][:, ic * P:(ic + 1) * P], rhs_x, False, False)
                mm(sl, xsq[0][:, ic * P:(ic + 1) * P], rhs_q, False, False)
                mm(sl, xsq[1][:, ic * P:(ic + 1) * P], rhs_q, False, True)
            nm_ps = psF.tile([1, N], f32, tag="F", name="nm_ps")
            for ih in range(2):
                io = ih * 512
                for cc in range(2):
                    mm(nm_ps[:, io:io + 512], negones, x_bf[b][cc][:, io:io + 512],
                       start=(cc == 0), stop=(cc == 1))
            return st_ps, nm_ps

        def _stats_post(b, st_ps, nm_ps):
            st_sb = tmp.tile([P, 8, 2], f32, tag="st_sb", name="st_sb")
            nc.vector.tensor_copy(out=st_sb[:], in_=st_ps[:])
            m2 = tmp.tile([P, 8], f32, tag="m2", name="m2")
            nc.vector.tensor_mul(m2[:], st_sb[:, :, 0], st_sb[:, :, 0])
            var = tmp.tile([P, 8], f32, tag="var", name="var")
            nc.vector.tensor_sub(var[:], st_sb[:, :, 1], m2[:])
            lnv = tmp.tile([P, 8], f32, tag="lnv", name="lnv")
            nc.scalar.activation(lnv[:], var[:], AF.Ln, bias=eps_col[:])
            nc.scalar.activation(rstd[b][:], lnv[:], AF.Exp, scale=-0.5)
            nc.scalar.copy(out=negmu[b][:], in_=nm_ps[:])

        def _qkv(b, ot):
            # ot 0-1: q tiles, ot 2-3: k tiles
            qk_ps = psF.tile([P, N], f32, tag="F", name="qk_ps")
            osl = slice(ot * P, (ot + 1) * P)
            for cc in range(2):
                for ih in range(2):
                    io = ih * 512
                    mm(qk_ps[:, io:io + 512], wqk_sb[:, cc, osl],
                       x_bf[b][cc][:, io:io + 512],
                       start=(cc == 0), stop=False)
            for ih in range(2):
                io = ih * 512
                mm(qk_ps[:, io:io + 512], wsqk_sb[:, osl], negmu[b][:, io:io + 512],
                   start=False, stop=True)
            dst = q_sb[b][ot] if ot < 2 else k_sb[b][ot - 2]
            nc.scalar.copy(out=dst[:], in_=qk_ps[:])

        def _vt(b, jc):
            vt_ps = psF.tile([P, 256], f32, tag="F", name="vt_ps")
            jsl = slice(jc * P, (jc + 1) * P)
            mm(vt_ps[:], x_bf[b][0][:, jsl], wvT_sb[:, 0, :], True, False)
            mm(vt_ps[:], x_bf[b][1][:, jsl], wvT_sb[:, 1, :], False, False)
            mm(vt_ps[:], negmu[b][:, jsl], wvs_sb[:], False, True)
            dst = vaug[b][:, jc // 2, jc % 2, :, 0:D]
            nc.vector.tensor_scalar_mul(
                dst, vt_ps.rearrange("p (h d) -> p h d", h=HEADS),
                rstd[b][:, jc:jc + 1])

        def _norms_a(b):
            qsq = [tmp.tile([P, N], bf16, tag=f"qsq{cc}", name=f"qsq{b}{cc}")
                   for cc in range(2)]
            for cc in range(2):
                nc.gpsimd.tensor_mul(qsq[cc][:], q_sb[b][cc][:], q_sb[b][cc][:])
            a_ln = tmp.tile([HEADS, N], f32, tag="a_ln", name="a_ln")
            s2q_ps = psF.tile([HEADS, N], f32, tag="F", name="s2q_ps")
            for ih in range(2):
                io = ih * 512
                for cc in range(2):
                    mm(s2q_ps[:, io:io + 512], E_ind[cc], qsq[cc][:, io:io + 512],
                       start=(cc == 0), stop=(cc == 1))
            nc.scalar.activation(a_ln[:], s2q_ps[:], AF.Ln)
            nc.scalar.activation(a_sb[b][:], a_ln[:], AF.Exp, scale=-0.5)
            for cc in range(2):
                for ih in range(2):
                    io = ih * 512
                    abc_ps = psF.tile([P, 512], f32, tag="F", name="abc_ps")
                    mm(abc_ps[:], E4[cc], a_sb[b][:, io:io + 512], True, True)
                    nc.vector.tensor_mul(q_sb[b][cc][:, io:io + 512],
                                         q_sb[b][cc][:, io:io + 512], abc_ps[:])

        def _norms_b(b):
            ksq = [tmp.tile([P, N], bf16, tag=f"ksq{cc}", name=f"ksq{b}{cc}")
                   for cc in range(2)]
            for cc in range(2):
                nc.gpsimd.tensor_mul(ksq[cc][:], k_sb[b][cc][:], k_sb[b][cc][:])
            bsq_ps = psF.tile([P, 8, HEADS], f32, tag="F", name="bsq_ps")
            for jc in range(8):
                for h in range(HEADS):
                    cc, hh = h // 2, h % 2
                    mm(bsq_ps[:, jc, h:h + 1], ksq[cc][:, jc * P:(jc + 1) * P],
                       khalf[hh], True, True)
            b8ln = tmp.tile([P, 8, HEADS], f32, tag="b8ln", name="b8ln")
            nc.scalar.activation(b8ln[:], bsq_ps[:], AF.Ln)
            nc.scalar.activation(b8e[b][:], b8ln[:], AF.Exp, scale=-0.5,
                                 bias=log8_col[:])
            nc.scalar.activation(b8s[b][:], b8ln[:], AF.Exp, scale=-0.5,
                                 bias=log8a8_col[:])

        def _norms(b):
            _norms_a(b)
            _norms_b(b)

        def _att(b, cc, ih, steps):
            hA, hB = 2 * cc, 2 * cc + 1
            io = ih * 512
            U2 = psU.tile([P, 2, 512], f32, tag="U", name=f"U{b}{cc}{ih}", bufs=1)
            u_ps[(b, cc, ih)] = U2
            for p in range(4):
                etA = etp.tile([P, 2, 512], fp8, tag="etA", name=f"etA{b}{cc}{ih}{p}")
                etB = etp.tile([P, 2, 512], fp8, tag="etB", name=f"etB{b}{cc}{ih}{p}")
                for s in range(2):
                    jc = 2 * p + s
                    jsl = slice(jc * P, (jc + 1) * P)
                    simA = psS.tile([P, 512], f32, tag="S", name="simA")
                    simB = psS.tile([P, 512], f32, tag="S", name="simB")
                    mm(simA[:], k_sb[b][cc][0:D, jsl],
                       q_sb[b][cc][0:D, io:io + 512], True, True)
                    mm(simB[:], k_sb[b][cc][D:P, jsl],
                       q_sb[b][cc][D:P, io:io + 512], True, True)
                    nc.scalar.activation(etA[:, s, :], simA[:], AF.Exp,
                                         bias=ce_col[:],
                                         scale=b8e[b][:, jc, hA:hA + 1])
                    nc.vector._custom_dve(SCH_EXP8, out=etB.bitcast(u8)[:, s, :],
                                          in0=simB[:],
                                          s0=b8s[b][:, jc, hB:hB + 1], s1=B8P)
                    stepi = steps.pop(0) if steps else None
                    if stepi is not None:
                        stepi()
                mm(U2[:, 0, :], vaug[b][:, p, :, hA, :], etA[:],
                   start=(p == 0), stop=(p == 3), perf_mode=DR)
                mm(U2[:, 1, :], vaug[b][:, p, :, hB, :], etB[:],
                   start=(p == 0), stop=(p == 3), perf_mode=DR)

        def _epi(b, cc, ih):
            ps = u_ps.pop((b, cc, ih))
            io = ih * 512
            den = tmp.tile([D, 2, 512], f32, tag="den", name=f"den{b}{cc}{ih}")
            nc.scalar.copy(out=den[:], in_=ps[D:P, :, :])
            nc.vector._custom_dve(RECIP1_MUL, out=u_sb[b][cc][0:D, :, io:io + 512],
                                  in0=den[:], in1=ps[0:D, :, :],
                                  s0=RC0, s1=RC1)

        def _proj(b, co):
            out_ps = psF.tile([P, N], f32, tag="F", name="out_ps")
            for ih in range(2):
                io = ih * 512
                for h in range(HEADS):
                    mm(out_ps[:, io:io + 512],
                       wot_sb[:, h, co * P:(co + 1) * P],
                       u_sb[b][h // 2][0:D, h % 2, io:io + 512],
                       start=(h == 0), stop=(h == 3))
            of = outf[b][co]
            nc.vector.tensor_add(of[:], out_ps[:], x_sb[b][co][:])
            nc.sync.dma_start(out_d[b, co * P:(co + 1) * P, :], of[:])

        outf = [[outp.tile([P, N], f32, tag="of", name=f"of{b}{co}")
                 for co in range(2)] for b in range(B)]

        # ---------------- schedule ----------------
        def _stats(b):
            _stats_post(b, *_stats_mm(b))

        _stats(0)
        for ot in range(4):
            _qkv(0, ot)
        for jc in range(8):
            _vt(0, jc)
        _norms(0)

        def _mk(f, *a):
            return lambda: f(*a)

        _att(0, 0, 0, [_mk(_stats, 1), _mk(_qkv, 1, 0), _mk(_qkv, 1, 1),
                       _mk(_qkv, 1, 2), _mk(_qkv, 1, 3), None, None, None])
        _att(0, 0, 1, [lambda: [_vt(1, j) for j in range(4)],
                       lambda: [_vt(1, j) for j in range(4, 8)],
                       None, None, None, None, None, None])
        _att(0, 1, 0, [_mk(_epi, 0, 0, 0), _mk(_epi, 0, 0, 1),
                       _mk(_norms_a, 1), None, None, None, None, None])
        _att(0, 1, 1, [_mk(_norms_b, 1), None, None, None,
                       None, None, None, None])
        _att(1, 0, 0, [_mk(_epi, 0, 1, 0), _mk(_epi, 0, 1, 1),
                       _mk(_proj, 0, 0), None, None, None, None, None])
        _att(1, 0, 1, [_mk(_proj, 0, 1), None, None, None,
                       None, None, None, None])
        _att(1, 1, 0, [_mk(_epi, 1, 0, 0), _mk(_epi, 1, 0, 1),
                       None, None, None, None, None, None])
        _att(1, 1, 1, [None, None, None, None, None, None, None, None])
        _epi(1, 1, 0)
        _epi(1, 1, 1)
        for co in range(2):
            _proj(1, co)

    nc.compile()
    return nc


_NC = None
last_exec_time_ns = None


def _get_nc():
    global _NC
    if _NC is None:
        _NC = build_nc()
    return _NC


def _run(in_maps, trace=False):
    global last_exec_time_ns
    from concourse.bass_utils import run_bass_kernel_spmd
    nc = _get_nc()
    if trace:
        _install_ntff_hook()
    try:
        res = run_bass_kernel_spmd(nc, in_maps, core_ids=list(range(NCORES)),
                                   trace=trace)
    except Exception:
        if not trace:
            raise
        res = run_bass_kernel_spmd(nc, in_maps, core_ids=list(range(NCORES)),
                                   trace=False)
    last_exec_time_ns = res.exec_time_ns
    return res


def kernel(x, g, w_qkv, w_out, _trace=False):
    import ml_dtypes
    x = np.ascontiguousarray(np.asarray(x, dtype=np.float32))
    g = np.asarray(g, dtype=np.float32).reshape(C)
    wqk, wvT, wsqk, wvs, wot = _host_weights(w_qkv, w_out, g)
    b_full, c, H, W = x.shape
    assert (b_full, c, H * W) == (NCORES * B, C, N)
    xr = x.reshape(b_full, C, N)
    cst, cst4 = _host_consts()
    in_maps = []
    for i in range(NCORES):
        xi = np.ascontiguousarray(xr[i * B:(i + 1) * B])
        in_maps.append({
            "x": xi,
            "xbf": xi.astype(ml_dtypes.bfloat16),
            "wqk": wqk,
            "wvT": wvT,
            "wsqk": wsqk,
            "wvs": wvs,
            "wot": wot,
            "cst": cst,
            "cst4": cst4,
        })
    res = _run(in_maps, trace=_trace)
    out = np.concatenate([res.results[i]["out"] for i in range(NCORES)], axis=0)
    return out.reshape(b_full, C, H, W).astype(np.float32)
